# revision 22
# baseline (speedup 1.0000x reference)
"""Multi-head attention (B=4, S=2048, E=1024, 16 heads, causal + padding
mask) on 8 Trainium2 NeuronCores.

Sharding: core = b*2 + g  (data parallel over batch b in 0..3, tensor
parallel over two groups g of 8 heads).  Each core computes q/k/v
projections for its 8 heads, full causal attention, and a partial output
projection over its 512 context channels; the host sums the two partial
outputs per batch (the TP all-reduce) and stacks batches.

Per core, matmul operands in fp32r (fp32 with 12 mantissa bits rounded
away — full-rate PE matmul dtype).  Scores are computed transposed
S^T[k, q] per head so softmax denominators come from an all-ones column
appended to V (M=65 matmuls); exp on ACT; causal mask via
gpsimd.affine_select; normalization after attn@V (commutes); padding
mask folded into V rows.  Next-pair K^T/Q^T projection chains and the
output projection are interleaved into the attention instruction stream
as PE filler so the tensor engine never idles (keeps the HAM clock
gate at 8/8 = 2.4 GHz).
"""

import os
import numpy as np

import concourse.bacc as bacc
import concourse.mybir as mybir
from concourse.tile import TileContext
from concourse.bass_utils import run_bass_kernel_spmd

F32 = mybir.dt.float32
F32R = mybir.dt.float32r
I32 = mybir.dt.int32

B, S, E = 4, 2048, 1024
NH, HD = 16, 64
NCORES = 8
SCALE = 1.0 / np.sqrt(HD)  # 0.125
NKB = S // 128  # 16 k-blocks
NCH = S // 512  # 4 q-chunks
NPAIR = 4  # head pairs per core (8 heads)

_CACHE = {}
LAST_RESULTS = None


def _round_f32r(x: np.ndarray) -> np.ndarray:
    """Round fp32 to fp32r (round-half-even, clearing 12 mantissa bits) —
    matches the hardware rounding exactly (verified on device)."""
    b = np.ascontiguousarray(x, dtype=np.float32).view(np.uint32)
    lsb = (b >> np.uint32(12)) & np.uint32(1)
    return ((b + np.uint32(0x7FF) + lsb) & np.uint32(0xFFFFF000)).view(np.float32)


def _build():
    nc = bacc.Bacc("TRN2", target_bir_lowering=False, debug=False)

    xT_d = nc.dram_tensor("xT", [E, S], F32, kind="ExternalInput")
    wq_d = nc.dram_tensor("wqT", [E, 512], F32, kind="ExternalInput")
    wk_d = nc.dram_tensor("wkT", [E, 512], F32, kind="ExternalInput")
    wv_d = nc.dram_tensor("wvT", [E, 512], F32, kind="ExternalInput")
    wo_d = nc.dram_tensor("woT", [512, E], F32, kind="ExternalInput")
    m_d = nc.dram_tensor("mask", [128, NKB], I32, kind="ExternalInput")
    o_d = nc.dram_tensor("o", [S, E], F32, kind="ExternalOutput")

    xT_r = xT_d[:].bitcast(F32R).rearrange("(o pi) f -> pi o f", pi=128)
    wq_r = wq_d[:].bitcast(F32R).rearrange("(o pi) f -> pi o f", pi=128)
    wk_r = wk_d[:].bitcast(F32R).rearrange("(o pi) f -> pi o f", pi=128)
    wv_r = wv_d[:].bitcast(F32R).rearrange("(o pi) f -> pi o f", pi=128)
    wo_r = wo_d[:].bitcast(F32R).rearrange("(o pi) f -> pi o f", pi=128)

    EXP = mybir.ActivationFunctionType.Exp
    MUL = mybir.AluOpType.mult

    with TileContext(nc) as tc:
        with (
            tc.tile_pool(name="big", bufs=15) as big,       # [128,2048] f32r slots
            tc.tile_pool(name="vp", bufs=16) as vp,         # V tiles [128,8,65]
            tc.tile_pool(name="wp", bufs=3) as wp,          # weight slices (8KB)
            tc.tile_pool(name="ep", bufs=4) as ep,          # exp tiles [128,1024]
            tc.tile_pool(name="sp", bufs=2) as sp,          # sums/recip [1,512]
            tc.tile_pool(name="bp", bufs=2) as bp,          # bcast [64,512]
            tc.tile_pool(name="gp", bufs=1) as gp,          # head-B stage [64,512]
            tc.tile_pool(name="cx", bufs=2) as cx,          # ctx sbuf [65,512]
            tc.tile_pool(name="mp", bufs=1) as mp,          # mask tiles
            tc.tile_pool(name="psF", bufs=3, space="PSUM") as psF,  # [128,1024]
            tc.tile_pool(name="psC", bufs=2, space="PSUM") as psC,  # [65,512]
        ):
            # padding mask -> float 0/1 per key position (k = kb*128 + p)
            mask_t = mp.tile([128, NKB], I32)
            nc.sync.dma_start(mask_t[:], m_d[:])
            pad01 = mp.tile([128, NKB], F32)
            nc.vector.tensor_copy(pad01[:], mask_t[:])

            # first V weight half, then x^T chunks in column halves (so the
            # PE can start V-block accumulation early), then the rest
            wv_t = []
            t = wp.tile([128, 4, 512], F32R, tag="w", name="wv0")
            nc.sync.dma_start(t[:], wv_r[:, 0:4, :])
            wv_t.append(t)
            xt = []
            for e in range(8):
                t = big.tile([128, S], F32R, tag="big", name=f"xt{e}")
                nc.sync.dma_start(t[:, 0:1024], xT_r[:, e, 0:1024])
                xt.append(t)
            t = wp.tile([128, 4, 512], F32R, tag="w", name="wv1")
            nc.sync.dma_start(t[:], wv_r[:, 4:8, :])
            wv_t.append(t)

            wqk = {}

            def load_wqk(p):
                t = wp.tile([128, 16, 128], F32R, tag="w", name=f"wqk{p}")
                nc.sync.dma_start(t[:, 0:8, :], wk_r[:, :, p * 128:(p + 1) * 128])
                nc.sync.dma_start(t[:, 8:16, :], wq_r[:, :, p * 128:(p + 1) * 128])
                wqk[p] = t

            load_wqk(0)
            for e in range(8):
                nc.sync.dma_start(xt[e][:, 1024:2048], xT_r[:, e, 1024:2048])

            kt = {}
            qt = {}
            cn = {}
            wo_t = []

            def emit_proj_chunk(which, p, ch):
                """One K^T or Q^T projection chain: 8 matmuls + copy."""
                dst = kt if which == "k" else qt
                if p not in dst:
                    dst[p] = big.tile(
                        [128, S], F32R, tag="big", name=f"{which}t{p}"
                    )
                base = 0 if which == "k" else 8
                ps = psF.tile([128, 1024], F32, tag="f", name=f"{which}tps")
                for e in range(8):
                    nc.tensor.matmul(
                        ps[:, 0:512],
                        wqk[p][:, base + e, :],
                        xt[e][:, ch * 512:(ch + 1) * 512],
                        start=(e == 0), stop=(e == 7),
                    )
                nc.vector.tensor_copy(
                    dst[p][:, ch * 512:(ch + 1) * 512], ps[:, 0:512]
                )


            # ---- V phase: V[s, (h,d)] + ones column, padding-masked ----
            v_t = [
                vp.tile([128, 8, 65], F32R, tag="v", name=f"v{sb}")
                for sb in range(NKB)
            ]
            def emit_v_block(sb):
                ps = psF.tile([128, 1024], F32, tag="f", name="vps")
                for e in range(8):
                    nc.tensor.matmul(
                        ps[:, 0:512],
                        xt[e][:, sb * 128:(sb + 1) * 128],
                        wv_t[e // 4][:, e % 4, :],
                        start=(e == 0), stop=(e == 7),
                    )
                # copy + padding mask in one op (psum f32 -> sbuf f32r)
                nc.vector.tensor_scalar(
                    v_t[sb][:, :, 0:64],
                    ps[:, 0:512], pad01[:, sb:sb + 1], None, MUL,
                )
            for sb in range(8):
                emit_v_block(sb)
            for sb in range(8):
                # ones column (padding-masked) for softmax denominators
                nc.vector.tensor_copy(
                    v_t[sb][:, :, 64],
                    pad01[:, sb:sb + 1].to_broadcast((128, 8)),
                )
            # first-column-half projection work while the second halves of
            # x^T are still streaming in; V blocks 8-15 and the second-half
            # projection chunks are deferred into pair-0's attention stream
            for ch in (0, 1):
                emit_proj_chunk("k", 0, ch)
                emit_proj_chunk("q", 0, ch)

            def deferred_startup(c):
                lo = 8 if c == 0 else 12
                for sb in range(lo, lo + 4):
                    emit_v_block(sb)
                for sb in range(lo, lo + 4):
                    nc.vector.tensor_copy(
                        v_t[sb][:, :, 64],
                        pad01[:, sb:sb + 1].to_broadcast((128, 8)),
                    )
                emit_proj_chunk("k", 0, c + 2)
                emit_proj_chunk("q", 0, c + 2)

            def emit_oproj_sblock(sb):
                pso = psF.tile([128, 1024], F32, tag="f", name="ops")
                for eo in range(2):
                    for p in range(NPAIR):
                        nc.tensor.matmul(
                            pso[:, eo * 512:(eo + 1) * 512],
                            cn[p][:, sb * 128:(sb + 1) * 128],
                            wo_t[eo][:, p, :],
                            start=(p == 0), stop=(p == 3),
                        )
                ost = big.tile([128, E], F32, tag="big", name=f"ost{sb}")
                nc.vector.tensor_copy(ost[:], pso[:])
                nc.gpsimd.dma_start(o_d[sb * 128:(sb + 1) * 128, :], ost[:])

            def emit_attention_chunk(p, c, drip=None):
                def _drip():
                    if drip is not None:
                        drip()

                cw = slice(c * 512, (c + 1) * 512)
                nkb = 4 * c + 4
                heads = (
                    (0, slice(0, 64), (0, 0)),     # head A of pair
                    (1, slice(64, 128), (64, 0)),  # head B of pair
                )
                cps = [
                    psC.tile([65, 512], F32, tag="c", name=f"ctx{h}")
                    for h in range(2)
                ]
                # full blocks, fused in pairs of two k-blocks
                for f in range(2 * c):
                    kb0 = 2 * f
                    for h, rows, tp in heads:
                        psf = psF.tile([128, 1024], F32, tag="f", name="scf")
                        for j in range(2):
                            kb = kb0 + j
                            nc.tensor.matmul(
                                psf[:, j * 512:(j + 1) * 512],
                                kt[p][rows, kb * 128:(kb + 1) * 128],
                                qt[p][rows, cw],
                                start=True, stop=True,
                                tile_position=tp,
                            )
                        ex = ep.tile([128, 1024], F32R, tag="e", name="exf")
                        nc.scalar.activation(ex[:], psf[:], EXP, scale=SCALE)
                        for j in range(2):
                            kb = kb0 + j
                            nc.tensor.matmul(
                                cps[h][:],
                                v_t[kb][:, 2 * p + h, :],
                                ex[:, j * 512:(j + 1) * 512],
                                start=(kb == 0), stop=False,
                                skip_group_check=True,
                            )
                        _drip()
                # diagonal blocks (causal-partial), fused in pairs:
                # dpair 0 = (r0, r1), dpair 1 = (r2, r3); one exp each
                for dp_ in range(2):
                    r_a, r_b = 2 * dp_, 2 * dp_ + 1
                    for h, rows, tp in heads:
                        psd = psF.tile([128, 1024], F32, tag="f", name="scd")
                        for j, r in ((0, r_a), (1, r_b)):
                            kb = 4 * c + r
                            nc.tensor.matmul(
                                psd[:, j * 512:(j + 1) * 512],
                                kt[p][rows, kb * 128:(kb + 1) * 128],
                                qt[p][rows, cw],
                                start=True, stop=True,
                                tile_position=tp,
                            )
                        ex = ep.tile([128, 1024], F32R, tag="e", name="exd")
                        if dp_ == 0:
                            # r0 valid psd[0:512] -> ex[0:512];
                            # r1 valid psd[640:1024] -> ex[640:1024]
                            nc.scalar.activation(
                                ex[:], psd[:], EXP, scale=SCALE
                            )
                            spans = ((r_a, 0, 512), (r_b, 640, 384))
                        else:
                            # r2 valid psd[256:512] -> ex[0:256];
                            # r3 valid psd[896:1024] -> ex[640:768]
                            nc.scalar.activation(
                                ex[:, 0:768], psd[:, 256:1024], EXP,
                                scale=SCALE,
                            )
                            spans = ((r_a, 0, 256), (r_b, 640, 128))
                        for r, exoff, wdt in spans:
                            kb = 4 * c + r
                            # causal: keep where local_col >= partition
                            nc.gpsimd.affine_select(
                                out=ex[:, exoff:exoff + wdt],
                                in_=ex[:, exoff:exoff + wdt],
                                compare_op=mybir.AluOpType.is_ge,
                                fill=0.0, base=0, channel_multiplier=-1,
                                pattern=[[1, wdt]],
                            )
                            nc.tensor.matmul(
                                cps[h][:, 512 - wdt:512],
                                v_t[kb][:, 2 * p + h, :],
                                ex[:, exoff:exoff + wdt],
                                start=(kb == 0), stop=(kb == nkb - 1),
                                skip_group_check=True,
                            )
                        _drip()
                # drain ctx psum to SBUF fast, then normalize from SBUF
                for h, rows, tp in heads:
                    cxs = cx.tile([65, 512], F32, tag="x", name="cxs")
                    nc.vector.tensor_copy(cxs[:], cps[h][:])
                    # move sums row to partition 0 (custom DVE ops need it)
                    s0 = sp.tile([1, 512], F32, tag="s", name="s0")
                    nc.gpsimd.dma_start(s0[:], cxs[64:65, :])
                    recip = sp.tile([1, 512], F32, tag="s", name="recip")
                    nc.vector.reciprocal_approx_fast(recip[:], s0[:])
                    bc = bp.tile([64, 512], F32, tag="b", name="bc")
                    nc.gpsimd.partition_broadcast(bc[:], recip[:])
                    if h == 0:
                        nc.vector.tensor_tensor(
                            cn[p][0:64, cw], cxs[0:64, :], bc[:], MUL
                        )
                    else:
                        stg = gp.tile([64, 512], F32R, tag="g", name="stg")
                        nc.vector.tensor_tensor(
                            stg[:], cxs[0:64, :], bc[:], MUL
                        )
                        # partition shift 0:64 -> 64:128 via SBUF DMA
                        nc.gpsimd.dma_start(cn[p][64:128, cw], stg[:])

            def proj_chunk_ops(which, p, ch):
                """Micro-ops (closures) of one K^T/Q^T chain: 8 mm + copy."""
                dst = kt if which == "k" else qt
                if p not in dst:
                    dst[p] = big.tile(
                        [128, S], F32R, tag="big", name=f"{which}t{p}"
                    )
                base = 0 if which == "k" else 8
                state = {}

                def mk_mm(e):
                    def op():
                        if "ps" not in state:
                            state["ps"] = psF.tile(
                                [128, 1024], F32, tag="f", name=f"{which}tps"
                            )
                        nc.tensor.matmul(
                            state["ps"][:, 0:512],
                            wqk[p][:, base + e, :],
                            xt[e][:, ch * 512:(ch + 1) * 512],
                            start=(e == 0), stop=(e == 7),
                        )
                    return op

                def cp():
                    nc.vector.tensor_copy(
                        dst[p][:, ch * 512:(ch + 1) * 512],
                        state["ps"][:, 0:512],
                    )

                return [mk_mm(e) for e in range(8)] + [cp]

            def oproj_sblock_ops(sb):
                state = {}

                def mk_mm(eo, p):
                    def op():
                        if "ps" not in state:
                            state["ps"] = psF.tile(
                                [128, 1024], F32, tag="f", name="ops"
                            )
                        nc.tensor.matmul(
                            state["ps"][:, eo * 512:(eo + 1) * 512],
                            cn[p][:, sb * 128:(sb + 1) * 128],
                            wo_t[eo][:, p, :],
                            start=(p == 0), stop=(p == 3),
                        )
                    return op

                def fin():
                    ost = big.tile([128, E], F32, tag="big", name=f"ost{sb}")
                    nc.vector.tensor_copy(ost[:], state["ps"][:])
                    nc.gpsimd.dma_start(o_d[sb * 128:(sb + 1) * 128, :], ost[:])

                return [mk_mm(eo, p) for eo in range(2) for p in range(NPAIR)] \
                    + [fin]



            # ---- attention per pair, with PE filler woven in ----
            for p in range(NPAIR):
                cn[p] = big.tile([128, S], F32R, tag="big", name=f"cn{p}")
                if p + 1 < NPAIR:
                    load_wqk(p + 1)
                if p == 2:
                    for eo in range(2):
                        t = wp.tile([128, 4, 512], F32R, tag="w", name=f"wo{eo}")
                        nc.sync.dma_start(
                            t[:], wo_r[:, :, eo * 512:(eo + 1) * 512]
                        )
                        wo_t.append(t)
                pending = []

                def drip():
                    if pending:
                        pending.pop(0)()

                # pair 0 ascends (chunks 0,1 run on first-column-half data
                # while x^T second halves stream in); pairs 1-3 descend so
                # the light chunks sit mid-stream and pair boundaries stay
                # PE-dense.  Filler entries are (target_pair, which, chunk)
                # projection chains, placed a pair-chunk before first use.
                if p == 0:
                    order = (0, 1, 2, 3)
                    sched = {2: [(1, "k", 0), (1, "k", 1), (1, "q", 3)],
                             3: [(1, "k", 2), (1, "k", 3), (1, "q", 2)]}
                elif p < NPAIR - 1:
                    order = (3, 2, 1, 0)
                    sched = {3: [(p, "q", 1), (p + 1, "k", 0), (p + 1, "k", 1)],
                             2: [(p, "q", 0), (p + 1, "k", 2)],
                             1: [(p + 1, "k", 3), (p + 1, "q", 3)],
                             0: [(p + 1, "q", 2)]}
                else:
                    order = (3, 2, 0, 1)
                    sched = {3: [(p, "q", 0)], 2: [(p, "q", 1)]}
                oproj_sched = {2: range(12, 16), 0: range(8, 12),
                               1: range(0, 4)}
                for c in order:
                    emit_attention_chunk(p, c, None)
                    if p == 0 and c in (0, 1):
                        deferred_startup(c)
                    for tp_, which, ch in sched.get(c, []):
                        pending += proj_chunk_ops(which, tp_, ch)
                    if p == NPAIR - 1:
                        # output projection as PE filler, lagged one chunk so
                        # it never waits on the just-finished normalize chain
                        for sb in oproj_sched.get(c, ()):
                            pending += oproj_sblock_ops(sb)
                    while pending:
                        pending.pop(0)()
                if p == NPAIR - 1:
                    for sb in range(4, 8):
                        for op in oproj_sblock_ops(sb):
                            op()

    nc.compile()
    return nc


def kernel(x, attention_mask, w_q, w_k, w_v, w_o):
    global LAST_RESULTS
    x = np.asarray(x, dtype=np.float32)
    attention_mask = np.asarray(attention_mask, dtype=np.int32)
    w_q = np.asarray(w_q, dtype=np.float32)
    w_k = np.asarray(w_k, dtype=np.float32)
    w_v = np.asarray(w_v, dtype=np.float32)
    w_o = np.asarray(w_o, dtype=np.float32)

    if "nc" not in _CACHE:
        _CACHE["nc"] = _build()
    nc = _CACHE["nc"]

    # host-side shard prep (layout/transpose + fp32r rounding only)
    wqT = [_round_f32r(w_q[g * 512:(g + 1) * 512].T) for g in range(2)]
    wkT = [_round_f32r(w_k[g * 512:(g + 1) * 512].T) for g in range(2)]
    wvT = [_round_f32r(w_v[g * 512:(g + 1) * 512].T) for g in range(2)]
    woT = [_round_f32r(w_o[:, g * 512:(g + 1) * 512].T) for g in range(2)]
    in_maps = []
    for core in range(NCORES):
        b, g = core // 2, core % 2
        in_maps.append({
            "xT": _round_f32r(x[b].T),
            "wqT": wqT[g],
            "wkT": wkT[g],
            "wvT": wvT[g],
            "woT": woT[g],
            "mask": np.ascontiguousarray(
                attention_mask[b].reshape(NKB, 128).T
            ).astype(np.int32),
        })

    trace = os.environ.get("KERNEL_TRACE", "") == "1"
    LAST_RESULTS = run_bass_kernel_spmd(
        nc, in_maps, core_ids=list(range(NCORES)), trace=trace
    )
    res = LAST_RESULTS.results

    out = np.empty((B, S, E), dtype=np.float32)
    for b in range(B):
        out[b] = res[2 * b]["o"] + res[2 * b + 1]["o"]
    return out


# revision 23
# speedup vs baseline: 1.0882x; 1.0882x over previous
"""Multi-head attention (B=4, S=2048, E=1024, 16 heads, causal + padding
mask) on 8 Trainium2 NeuronCores.

Sharding: core = b*2 + g  (data parallel over batch b in 0..3, tensor
parallel over two groups g of 8 heads).  Each core computes q/k/v
projections for its 8 heads, full causal attention, and a partial output
projection over its 512 context channels; the host sums the two partial
outputs per batch (the TP all-reduce) and stacks batches.

Per core, matmul operands in fp32r (fp32 with 12 mantissa bits rounded
away — full-rate PE matmul dtype).  Scores are computed transposed
S^T[k, q] per head so softmax denominators come from an all-ones column
appended to V (M=65 matmuls); exp on ACT; causal mask via
gpsimd.affine_select; normalization after attn@V (commutes); padding
mask folded into V rows.  Next-pair K^T/Q^T projection chains and the
output projection are interleaved into the attention instruction stream
as PE filler so the tensor engine never idles (keeps the HAM clock
gate at 8/8 = 2.4 GHz).
"""

import os
import numpy as np

import concourse.bacc as bacc
import concourse.mybir as mybir
from concourse.tile import TileContext
from concourse.bass_utils import run_bass_kernel_spmd

F32 = mybir.dt.float32
F32R = mybir.dt.float32r
I32 = mybir.dt.int32

B, S, E = 4, 2048, 1024
NH, HD = 16, 64
NCORES = 8
SCALE = 1.0 / np.sqrt(HD)  # 0.125
NKB = S // 128  # 16 k-blocks
NCH = S // 512  # 4 q-chunks
NPAIR = 4  # head pairs per core (8 heads)

_CACHE = {}
LAST_RESULTS = None


def _round_f32r(x: np.ndarray) -> np.ndarray:
    """Round fp32 to fp32r (round-half-even, clearing 12 mantissa bits) —
    matches the hardware rounding exactly (verified on device)."""
    b = np.ascontiguousarray(x, dtype=np.float32).view(np.uint32)
    lsb = (b >> np.uint32(12)) & np.uint32(1)
    return ((b + np.uint32(0x7FF) + lsb) & np.uint32(0xFFFFF000)).view(np.float32)


def _build():
    nc = bacc.Bacc("TRN2", target_bir_lowering=False, debug=False)

    xT_d = nc.dram_tensor("xT", [E, S], F32, kind="ExternalInput")
    wq_d = nc.dram_tensor("wqT", [E, 512], F32, kind="ExternalInput")
    wk_d = nc.dram_tensor("wkT", [E, 512], F32, kind="ExternalInput")
    wv_d = nc.dram_tensor("wvT", [E, 512], F32, kind="ExternalInput")
    wo_d = nc.dram_tensor("woT", [512, E], F32, kind="ExternalInput")
    m_d = nc.dram_tensor("mask", [128, NKB], I32, kind="ExternalInput")
    o_d = nc.dram_tensor("o", [S, E], F32, kind="ExternalOutput")

    xT_r = xT_d[:].bitcast(F32R).rearrange("(o pi) f -> pi o f", pi=128)
    wq_r = wq_d[:].bitcast(F32R).rearrange("(o pi) f -> pi o f", pi=128)
    wk_r = wk_d[:].bitcast(F32R).rearrange("(o pi) f -> pi o f", pi=128)
    wv_r = wv_d[:].bitcast(F32R).rearrange("(o pi) f -> pi o f", pi=128)
    wo_r = wo_d[:].bitcast(F32R).rearrange("(o pi) f -> pi o f", pi=128)

    EXP = mybir.ActivationFunctionType.Exp
    MUL = mybir.AluOpType.mult

    with TileContext(nc) as tc:
        with (
            tc.tile_pool(name="big", bufs=15) as big,       # [128,2048] f32r slots
            tc.tile_pool(name="vp", bufs=16) as vp,         # V tiles [128,8,65]
            tc.tile_pool(name="wp", bufs=3) as wp,          # weight slices (8KB)
            tc.tile_pool(name="ep", bufs=4) as ep,          # exp tiles [128,1024]
            tc.tile_pool(name="sp", bufs=2) as sp,          # sums/recip [1,512]
            tc.tile_pool(name="bp", bufs=2) as bp,          # bcast [64,512]
            tc.tile_pool(name="gp", bufs=1) as gp,          # head-B stage [64,512]
            tc.tile_pool(name="cx", bufs=2) as cx,          # ctx sbuf [65,512]
            tc.tile_pool(name="mp", bufs=1) as mp,          # mask tiles
            tc.tile_pool(name="psF", bufs=3, space="PSUM") as psF,  # [128,1024]
            tc.tile_pool(name="psC", bufs=2, space="PSUM") as psC,  # [65,512]
        ):
            # padding mask -> float 0/1 per key position (k = kb*128 + p)
            mask_t = mp.tile([128, NKB], I32)
            nc.sync.dma_start(mask_t[:], m_d[:])
            pad01 = mp.tile([128, NKB], F32)
            nc.vector.tensor_copy(pad01[:], mask_t[:])

            # first V weight half, then x^T chunks in column halves (so the
            # PE can start V-block accumulation early), then the rest
            wv_t = []
            t = wp.tile([128, 4, 512], F32R, tag="w", name="wv0")
            nc.sync.dma_start(t[:], wv_r[:, 0:4, :])
            wv_t.append(t)
            xt = []
            for e in range(8):
                t = big.tile([128, S], F32R, tag="big", name=f"xt{e}")
                nc.sync.dma_start(t[:, 0:1024], xT_r[:, e, 0:1024])
                xt.append(t)
            t = wp.tile([128, 4, 512], F32R, tag="w", name="wv1")
            nc.sync.dma_start(t[:], wv_r[:, 4:8, :])
            wv_t.append(t)

            wqk = {}

            def load_wqk(p):
                t = wp.tile([128, 16, 128], F32R, tag="w", name=f"wqk{p}")
                nc.sync.dma_start(t[:, 0:8, :], wk_r[:, :, p * 128:(p + 1) * 128])
                nc.sync.dma_start(t[:, 8:16, :], wq_r[:, :, p * 128:(p + 1) * 128])
                wqk[p] = t

            load_wqk(0)
            for e in range(8):
                nc.sync.dma_start(xt[e][:, 1024:2048], xT_r[:, e, 1024:2048])

            kt = {}
            qt = {}
            cn = {}
            wo_t = []

            def emit_proj_chunk(which, p, ch):
                """One K^T or Q^T projection chain: 8 matmuls + copy."""
                dst = kt if which == "k" else qt
                if p not in dst:
                    dst[p] = big.tile(
                        [128, S], F32R, tag="big", name=f"{which}t{p}"
                    )
                base = 0 if which == "k" else 8
                ps = psF.tile([128, 1024], F32, tag="f", name=f"{which}tps")
                for e in range(8):
                    nc.tensor.matmul(
                        ps[:, 0:512],
                        wqk[p][:, base + e, :],
                        xt[e][:, ch * 512:(ch + 1) * 512],
                        start=(e == 0), stop=(e == 7),
                    )
                nc.vector.tensor_copy(
                    dst[p][:, ch * 512:(ch + 1) * 512], ps[:, 0:512]
                )


            # ---- V phase: V[s, (h,d)] + ones column, padding-masked ----
            v_t = [
                vp.tile([128, 8, 65], F32R, tag="v", name=f"v{sb}")
                for sb in range(NKB)
            ]
            def emit_v_block(sb):
                ps = psF.tile([128, 1024], F32, tag="f", name="vps")
                for e in range(8):
                    nc.tensor.matmul(
                        ps[:, 0:512],
                        xt[e][:, sb * 128:(sb + 1) * 128],
                        wv_t[e // 4][:, e % 4, :],
                        start=(e == 0), stop=(e == 7),
                    )
                # copy + padding mask in one op (psum f32 -> sbuf f32r)
                nc.vector.tensor_scalar(
                    v_t[sb][:, :, 0:64],
                    ps[:, 0:512], pad01[:, sb:sb + 1], None, MUL,
                )
            for sb in range(8):
                emit_v_block(sb)
            for sb in range(8):
                # ones column (padding-masked) for softmax denominators
                nc.vector.tensor_copy(
                    v_t[sb][:, :, 64],
                    pad01[:, sb:sb + 1].to_broadcast((128, 8)),
                )
            # first-column-half projection work while the second halves of
            # x^T are still streaming in; V blocks 8-15 and the second-half
            # projection chunks are deferred into pair-0's attention stream
            for ch in (0, 1):
                emit_proj_chunk("k", 0, ch)
                emit_proj_chunk("q", 0, ch)

            def deferred_startup(c):
                lo = 8 if c == 0 else 12
                for sb in range(lo, lo + 4):
                    emit_v_block(sb)
                for sb in range(lo, lo + 4):
                    nc.vector.tensor_copy(
                        v_t[sb][:, :, 64],
                        pad01[:, sb:sb + 1].to_broadcast((128, 8)),
                    )
                emit_proj_chunk("k", 0, c + 2)
                emit_proj_chunk("q", 0, c + 2)

            def emit_oproj_sblock(sb):
                pso = psF.tile([128, 1024], F32, tag="f", name="ops")
                for eo in range(2):
                    for p in range(NPAIR):
                        nc.tensor.matmul(
                            pso[:, eo * 512:(eo + 1) * 512],
                            cn[p][:, sb * 128:(sb + 1) * 128],
                            wo_t[eo][:, p, :],
                            start=(p == 0), stop=(p == 3),
                        )
                ost = big.tile([128, E], F32, tag="big", name=f"ost{sb}")
                nc.vector.tensor_copy(ost[:], pso[:])
                nc.gpsimd.dma_start(o_d[sb * 128:(sb + 1) * 128, :], ost[:])

            def emit_attention_chunk(p, c, drip=None):
                def _drip():
                    if drip is not None:
                        drip()

                cw = slice(c * 512, (c + 1) * 512)
                nkb = 4 * c + 4
                heads = (
                    (0, slice(0, 64), (0, 0)),     # head A of pair
                    (1, slice(64, 128), (64, 0)),  # head B of pair
                )
                cps = [
                    psC.tile([65, 512], F32, tag="c", name=f"ctx{h}")
                    for h in range(2)
                ]
                # full blocks, fused in pairs of two k-blocks
                for f in range(2 * c):
                    kb0 = 2 * f
                    for h, rows, tp in heads:
                        psf = psF.tile([128, 1024], F32, tag="f", name="scf")
                        for j in range(2):
                            kb = kb0 + j
                            nc.tensor.matmul(
                                psf[:, j * 512:(j + 1) * 512],
                                kt[p][rows, kb * 128:(kb + 1) * 128],
                                qt[p][rows, cw],
                                start=True, stop=True,
                                tile_position=tp,
                            )
                        ex = ep.tile([128, 1024], F32R, tag="e", name="exf")
                        nc.scalar.activation(ex[:], psf[:], EXP, scale=SCALE)
                        for j in range(2):
                            kb = kb0 + j
                            nc.tensor.matmul(
                                cps[h][:],
                                v_t[kb][:, 2 * p + h, :],
                                ex[:, j * 512:(j + 1) * 512],
                                start=(kb == 0), stop=False,
                                skip_group_check=True,
                            )
                        _drip()
                # diagonal blocks (causal-partial), fused in pairs:
                # dpair 0 = (r0, r1), dpair 1 = (r2, r3); one exp each
                for dp_ in range(2):
                    r_a, r_b = 2 * dp_, 2 * dp_ + 1
                    for h, rows, tp in heads:
                        psd = psF.tile([128, 1024], F32, tag="f", name="scd")
                        for j, r in ((0, r_a), (1, r_b)):
                            kb = 4 * c + r
                            nc.tensor.matmul(
                                psd[:, j * 512:(j + 1) * 512],
                                kt[p][rows, kb * 128:(kb + 1) * 128],
                                qt[p][rows, cw],
                                start=True, stop=True,
                                tile_position=tp,
                            )
                        ex = ep.tile([128, 1024], F32R, tag="e", name="exd")
                        if dp_ == 0:
                            # r0 valid psd[0:512] -> ex[0:512];
                            # r1 valid psd[640:1024] -> ex[640:1024]
                            nc.scalar.activation(
                                ex[:], psd[:], EXP, scale=SCALE
                            )
                            spans = ((r_a, 0, 512), (r_b, 640, 384))
                        else:
                            # r2 valid psd[256:512] -> ex[0:256];
                            # r3 valid psd[896:1024] -> ex[640:768]
                            nc.scalar.activation(
                                ex[:, 0:768], psd[:, 256:1024], EXP,
                                scale=SCALE,
                            )
                            spans = ((r_a, 0, 256), (r_b, 640, 128))
                        for r, exoff, wdt in spans:
                            kb = 4 * c + r
                            # causal: keep where local_col >= partition
                            nc.gpsimd.affine_select(
                                out=ex[:, exoff:exoff + wdt],
                                in_=ex[:, exoff:exoff + wdt],
                                compare_op=mybir.AluOpType.is_ge,
                                fill=0.0, base=0, channel_multiplier=-1,
                                pattern=[[1, wdt]],
                            )
                            nc.tensor.matmul(
                                cps[h][:, 512 - wdt:512],
                                v_t[kb][:, 2 * p + h, :],
                                ex[:, exoff:exoff + wdt],
                                start=(kb == 0), stop=(kb == nkb - 1),
                                skip_group_check=True,
                            )
                        _drip()
                # drain ctx psum to SBUF fast, then normalize from SBUF
                for h, rows, tp in heads:
                    cxs = cx.tile([65, 512], F32, tag="x", name="cxs")
                    nc.vector.tensor_copy(cxs[:], cps[h][:])
                    # move sums row to partition 0 (custom DVE ops need it)
                    s0 = sp.tile([1, 512], F32, tag="s", name="s0")
                    nc.gpsimd.dma_start(s0[:], cxs[64:65, :])
                    recip = sp.tile([1, 512], F32, tag="s", name="recip")
                    nc.vector.reciprocal_approx_fast(recip[:], s0[:])
                    bc = bp.tile([64, 512], F32, tag="b", name="bc")
                    nc.gpsimd.partition_broadcast(bc[:], recip[:])
                    if h == 0:
                        nc.vector.tensor_tensor(
                            cn[p][0:64, cw], cxs[0:64, :], bc[:], MUL
                        )
                    else:
                        stg = gp.tile([64, 512], F32R, tag="g", name="stg")
                        nc.vector.tensor_tensor(
                            stg[:], cxs[0:64, :], bc[:], MUL
                        )
                        # partition shift 0:64 -> 64:128 via SBUF DMA
                        nc.gpsimd.dma_start(cn[p][64:128, cw], stg[:])

            def proj_chunk_ops(which, p, ch):
                """Micro-ops (closures) of one K^T/Q^T chain: 8 mm + copy."""
                dst = kt if which == "k" else qt
                if p not in dst:
                    dst[p] = big.tile(
                        [128, S], F32R, tag="big", name=f"{which}t{p}"
                    )
                base = 0 if which == "k" else 8
                state = {}

                def mk_mm(e):
                    def op():
                        if "ps" not in state:
                            state["ps"] = psF.tile(
                                [128, 1024], F32, tag="f", name=f"{which}tps"
                            )
                        nc.tensor.matmul(
                            state["ps"][:, 0:512],
                            wqk[p][:, base + e, :],
                            xt[e][:, ch * 512:(ch + 1) * 512],
                            start=(e == 0), stop=(e == 7),
                        )
                    return op

                def cp():
                    nc.vector.tensor_copy(
                        dst[p][:, ch * 512:(ch + 1) * 512],
                        state["ps"][:, 0:512],
                    )

                return [mk_mm(e) for e in range(8)] + [cp]

            def oproj_sblock_ops(sb):
                state = {}

                def mk_mm(eo, p):
                    def op():
                        if "ps" not in state:
                            state["ps"] = psF.tile(
                                [128, 1024], F32, tag="f", name="ops"
                            )
                        nc.tensor.matmul(
                            state["ps"][:, eo * 512:(eo + 1) * 512],
                            cn[p][:, sb * 128:(sb + 1) * 128],
                            wo_t[eo][:, p, :],
                            start=(p == 0), stop=(p == 3),
                        )
                    return op

                def fin():
                    ost = big.tile([128, E], F32, tag="big", name=f"ost{sb}")
                    nc.vector.tensor_copy(ost[:], state["ps"][:])
                    nc.gpsimd.dma_start(o_d[sb * 128:(sb + 1) * 128, :], ost[:])

                return [mk_mm(eo, p) for eo in range(2) for p in range(NPAIR)] \
                    + [fin]



            # ---- attention per pair, with PE filler woven in ----
            for p in range(NPAIR):
                cn[p] = big.tile([128, S], F32R, tag="big", name=f"cn{p}")
                if p + 1 < NPAIR:
                    load_wqk(p + 1)
                if p == 2:
                    for eo in range(2):
                        t = wp.tile([128, 4, 512], F32R, tag="w", name=f"wo{eo}")
                        nc.sync.dma_start(
                            t[:], wo_r[:, :, eo * 512:(eo + 1) * 512]
                        )
                        wo_t.append(t)
                pending = []

                def drip():
                    if pending:
                        pending.pop(0)()

                # pair 0 ascends (chunks 0,1 run on first-column-half data
                # while x^T second halves stream in); pairs 1-3 descend so
                # the light chunks sit mid-stream and pair boundaries stay
                # PE-dense.  Filler entries are (target_pair, which, chunk)
                # projection chains, placed a pair-chunk before first use.
                if p == 0:
                    order = (0, 1, 2, 3)
                    sched = {2: [(1, "k", 0), (1, "k", 1), (1, "q", 3)],
                             3: [(1, "k", 2), (1, "k", 3), (1, "q", 2)]}
                elif p < NPAIR - 1:
                    order = (3, 2, 1, 0)
                    sched = {3: [(p, "q", 1), (p + 1, "k", 0), (p + 1, "k", 1)],
                             2: [(p, "q", 0), (p + 1, "k", 2)],
                             1: [(p + 1, "k", 3), (p + 1, "q", 3)],
                             0: [(p + 1, "q", 2)]}
                    if p == NPAIR - 2:
                        # pair 3 gets all its projections as pair-2 filler so
                        # x^T frees (8 big-pool slots) before pair 3 starts
                        sched[1].append((3, "q", 1))
                        sched[0].append((3, "q", 0))
                else:
                    order = (3, 2, 0, 1)
                    sched = {}
                oproj_sched = {2: range(12, 16), 0: range(8, 12),
                               1: range(0, 4)}
                for c in order:
                    emit_attention_chunk(p, c, None)
                    if p == 0 and c in (0, 1):
                        deferred_startup(c)
                    for tp_, which, ch in sched.get(c, []):
                        pending += proj_chunk_ops(which, tp_, ch)
                    if p == NPAIR - 1:
                        # output projection as PE filler, lagged one chunk so
                        # it never waits on the just-finished normalize chain
                        for sb in oproj_sched.get(c, ()):
                            pending += oproj_sblock_ops(sb)
                    while pending:
                        pending.pop(0)()
                if p == NPAIR - 1:
                    for sb in range(4, 8):
                        for op in oproj_sblock_ops(sb):
                            op()

    nc.compile()
    return nc


def kernel(x, attention_mask, w_q, w_k, w_v, w_o):
    global LAST_RESULTS
    x = np.asarray(x, dtype=np.float32)
    attention_mask = np.asarray(attention_mask, dtype=np.int32)
    w_q = np.asarray(w_q, dtype=np.float32)
    w_k = np.asarray(w_k, dtype=np.float32)
    w_v = np.asarray(w_v, dtype=np.float32)
    w_o = np.asarray(w_o, dtype=np.float32)

    if "nc" not in _CACHE:
        _CACHE["nc"] = _build()
    nc = _CACHE["nc"]

    # host-side shard prep (layout/transpose + fp32r rounding only)
    wqT = [_round_f32r(w_q[g * 512:(g + 1) * 512].T) for g in range(2)]
    wkT = [_round_f32r(w_k[g * 512:(g + 1) * 512].T) for g in range(2)]
    wvT = [_round_f32r(w_v[g * 512:(g + 1) * 512].T) for g in range(2)]
    woT = [_round_f32r(w_o[:, g * 512:(g + 1) * 512].T) for g in range(2)]
    in_maps = []
    for core in range(NCORES):
        b, g = core // 2, core % 2
        in_maps.append({
            "xT": _round_f32r(x[b].T),
            "wqT": wqT[g],
            "wkT": wkT[g],
            "wvT": wvT[g],
            "woT": woT[g],
            "mask": np.ascontiguousarray(
                attention_mask[b].reshape(NKB, 128).T
            ).astype(np.int32),
        })

    trace = os.environ.get("KERNEL_TRACE", "") == "1"
    LAST_RESULTS = run_bass_kernel_spmd(
        nc, in_maps, core_ids=list(range(NCORES)), trace=trace
    )
    res = LAST_RESULTS.results

    out = np.empty((B, S, E), dtype=np.float32)
    for b in range(B):
        out[b] = res[2 * b]["o"] + res[2 * b + 1]["o"]
    return out
